# revision 15
# baseline (speedup 1.0000x reference)
"""Trainium2 Bass kernel for nn_MEPG_Loss (MEPG policy-gradient loss).

Math (forward only; stop_gradient is identity):
    h   = tanh(states[s,:,t] @ W1 + b1)                  [S,T,H]
    mu  = h @ W2 + b2                                    [S,T,A]
    ll[s,t] = -0.5*(||a[s,:,t]-mu||^2/SD + A*log(2*pi*SD))
    out = sum_s (sum_t A_hat[t,s]) * (sum_t ll[t,s]) / S

Per-simulation reductions with v = W2^T h (device) and c = b2 - a (host):
    q_sum[s] = sum_t ||v+c||^2 = sum_t ||v||^2 + 2 sum_t <v,c> + sum_t ||c||^2
      - sum_t v, sum_t v^2 per partition: ONE bn_stats on the mu psum bank
      - cross = sum_t v*c per partition: ONE scalar_tensor_tensor accum
      - sum_t ||c||^2, rewards sums: host numpy (inputs are host-resident)
    q_last[s]: copy v[:, T-1], combine with host c[:, T-1]

Device pipeline, per core (256 sims as 64 quads of 4 sims):
    - states prepacked on host to [64, NQ*T] bf16; contiguous block DMAs
      (HWDGE via sync queue; gpsimd SWDGE costs ~600ns/DMA of Q7 time)
    - mm1: 4 row-tiled K=16 matmuls -> 2-bank psum units (3-unit rotation;
      the ACT is each unit's ONLY reader so mm1 runs 1.5 quads ahead)
    - ScalarE: one merged tanh over 2048 cols when the quad's two units are
      adjacent (2/3 of quads), else two 1024-col tanhs.  ScalarE is the
      bottleneck engine: ~1 elem/lane/cycle @ 1.2 GHz, ~127 us total.
    - mm2: 4 col-tiled matmuls (lhsT=W2) -> mu psum bank (x2 rotation)
    - DVE: stt cross-term + qlast copy + bn_stats + bn_aggr per quad
Final combine (tiny) in float64 on host.
"""

import os
import sys

import numpy as np

if not any(os.path.isdir(os.path.join(p, "concourse")) for p in sys.path if p):
    sys.path.insert(0, "/opt/trn_rl_repo")

import ml_dtypes

import concourse.bacc as bacc
import concourse.tile as tile
from concourse import mybir
from concourse.bass_utils import run_bass_kernel_spmd

# Problem constants (hardcoded per contract)
S, D, A, T, HID = 2048, 16, 4, 512, 128
N_CORES = 8
SS = S // N_CORES          # 256 sims per core
NQ = SS // 4               # 64 quads per core
SD_VAR = 0.04
ALPHA = 0.1
MAX_POSITION = 1.0

# DMA blocks of quads: small first blocks so the first mm1/tanh start early
BLOCK_SIZES = [1, 1, 2] + [4] * 15
BLOCK_Q0 = np.concatenate([[0], np.cumsum(BLOCK_SIZES)[:-1]]).tolist()
NBL = len(BLOCK_SIZES)
QUAD_BLOCK = []
for bi, (q0, nq) in enumerate(zip(BLOCK_Q0, BLOCK_SIZES)):
    QUAD_BLOCK += [bi] * nq

F32 = mybir.dt.float32
BF16 = mybir.dt.bfloat16
NP_BF16 = ml_dtypes.bfloat16


def _build_program():
    nc = bacc.Bacc("TRN2", target_bir_lowering=False, debug=False)

    stp_d = nc.dram_tensor("st_pre", [64, NQ * T], BF16, kind="ExternalInput").ap()
    atp_d = nc.dram_tensor("at_pre", [16, NQ * T], BF16, kind="ExternalInput").ap()
    w1f_d = nc.dram_tensor("w1full", [128, HID], BF16, kind="ExternalInput").ap()
    w2_d = nc.dram_tensor("w2", [HID, A], BF16, kind="ExternalInput").ap()
    b1_d = nc.dram_tensor("b1col", [HID, 1], F32, kind="ExternalInput").ap()

    mv_d = nc.dram_tensor("mv", [128, 2 * NQ], F32, kind="ExternalOutput").ap()
    ql_d = nc.dram_tensor("ql", [128, NQ], F32, kind="ExternalOutput").ap()
    cr_d = nc.dram_tensor("cr", [128, NQ], F32, kind="ExternalOutput").ap()

    with tile.TileContext(nc) as tc:
        with (
            tc.tile_pool(name="consts", bufs=1) as consts,
            tc.tile_pool(name="stp", bufs=4) as stp,
            tc.tile_pool(name="atp", bufs=4) as atp,
            tc.tile_pool(name="hsb", bufs=2) as hsb,
            tc.tile_pool(name="sdp", bufs=2) as sdp,
            tc.tile_pool(name="bstp", bufs=2) as bstp,
            tc.tile_pool(name="outs", bufs=1) as outp,
            tc.tile_pool(name="hpp", bufs=1, space="PSUM") as hpp,
        ):
            # One persistent PSUM tile covering all 8 banks, hand-carved:
            # 3 h_pre units of 2 banks + 2 mu banks (1 bank spare)
            PS = hpp.tile([128, 4096], F32, tag="PS")
            UN = [PS[:, 1024 * u:1024 * (u + 1)] for u in range(3)]
            MU = [PS[:, 3072:3584], PS[:, 3584:4096]]

            # w1 + the first data block go on the DMA queues before anything
            # else so mm1(0) can start ASAP
            w1t = consts.tile([128, HID], BF16, tag="w1t")
            w2t = consts.tile([HID, A], BF16, tag="w2t")
            b1t = consts.tile([HID, 1], F32, tag="b1t")
            nc.sync.dma_start(out=w1t[:], in_=w1f_d)

            mv_sb = outp.tile([128, 2 * NQ], F32, tag="mv")
            ql_sb = outp.tile([128, NQ], F32, tag="ql")
            cr_sb = outp.tile([128, NQ], F32, tag="cr")

            blocks = {}

            def _ensure_block(bi):
                if bi in blocks or bi >= NBL:
                    return
                q0, nq = BLOCK_Q0[bi], BLOCK_SIZES[bi]
                c0 = T * q0
                st = stp.tile([128, nq * T], BF16, tag=f"st{nq}",
                              name=f"st_{bi}")
                at = atp.tile([128, nq * T], BF16, tag=f"at{nq}",
                              name=f"at_{bi}")
                for j in range(4):
                    nc.sync.dma_start(
                        out=st[32 * j:32 * j + D, :],
                        in_=stp_d[D * j:D * (j + 1), c0:c0 + nq * T],
                    )
                for j in range(4):
                    nc.sync.dma_start(
                        out=at[32 * j:32 * j + A, :],
                        in_=atp_d[A * j:A * (j + 1), c0:c0 + nq * T],
                    )
                blocks[bi] = (st, at)

            def _mm1u(u):
                # half-quad unit u = (quad u//2, sims j0..j0+1)
                g = u // 2
                bi = QUAD_BLOCK[g]
                st, _ = blocks[bi]
                q = g - BLOCK_Q0[bi]
                j0 = 2 * (u % 2)
                unit = UN[u % 3]
                for j in (j0, j0 + 1):
                    nc.tensor.matmul(
                        out=unit[:, T * (j - j0):T * (j - j0 + 1)],
                        lhsT=w1t[32 * j:32 * j + D, :],
                        rhs=st[32 * j:32 * j + D, T * q:T * (q + 1)],
                        start=True, stop=True,
                        tile_position=(32 * j, 0),
                    )

            hcur = [None]

            def _act_u(u):
                if u % 2 == 0:
                    hcur[0] = hsb.tile([128, 4 * T], BF16, tag="h",
                                       name=f"h_{u // 2}")
                h = hcur[0]
                half = u % 2
                nc.scalar.activation(
                    out=h[:, 2 * T * half:2 * T * (half + 1)], in_=UN[u % 3],
                    func=mybir.ActivationFunctionType.Tanh,
                    bias=b1t[:], scale=1.0,
                )
                return h

            def _tail_quad(g, h):
                bi = QUAD_BLOCK[g]
                _, at = blocks[bi]
                q = g - BLOCK_Q0[bi]
                mu = MU[g % 2]
                for j in range(4):
                    nc.tensor.matmul(
                        out=mu[32 * j:32 * j + A, :],
                        lhsT=w2t[:],
                        rhs=h[:, T * j:T * (j + 1)],
                        start=True, stop=True,
                        tile_position=(0, 32 * j),
                        skip_group_check=True,
                    )
                # cross = sum_t v*c per partition (c = b2-a, bf16)
                sd = sdp.tile([128, T], BF16, tag="sd", name=f"sd_{g}")
                nc.vector.scalar_tensor_tensor(
                    out=sd[:], in0=mu[:], scalar=1.0,
                    in1=at[:, T * q:T * (q + 1)],
                    op0=mybir.AluOpType.mult, op1=mybir.AluOpType.mult,
                    accum_out=cr_sb[:, g:g + 1],
                )
                nc.vector.tensor_copy(ql_sb[:, g:g + 1], mu[:, T - 1:T])
                sts = bstp.tile([128, 6], F32, tag="bst", name=f"bst_{g}")
                nc.vector.bn_stats(out=sts[:], in_=mu[:])
                nc.vector.bn_aggr(out=mv_sb[:, 2 * g:2 * g + 2], in_=sts[:])

            _ensure_block(0)
            _ensure_block(1)
            # dummy activation: forces the tanh table load at t~0
            dums = consts.tile([128, 1], F32, tag="dums")
            dumo = consts.tile([128, 1], F32, tag="dumo")
            nc.vector.memset(dums[:], 0.0)
            nc.scalar.activation(
                out=dumo[:], in_=dums[:],
                func=mybir.ActivationFunctionType.Tanh, scale=1.0,
            )
            nc.sync.dma_start(out=w2t[:], in_=w2_d)
            nc.sync.dma_start(out=b1t[:], in_=b1_d)
            _ensure_block(2)
            _ensure_block(3)

            # uniform half-quad pipeline: mm1 runs exactly 2 units ahead
            # (its psum unit is freed by ACT(u-1), i.e. right when ACT(u)
            # starts), ACTs run back-to-back, quad tails slot in behind
            NU = 2 * NQ
            _mm1u(0)
            _mm1u(1)
            for u in range(NU):
                if u + 2 < NU:
                    g2 = (u + 2) // 2
                    if (u + 2) % 2 == 0 and g2 == BLOCK_Q0[QUAD_BLOCK[g2]]:
                        _ensure_block(QUAD_BLOCK[g2] + 3)
                    _mm1u(u + 2)
                h = _act_u(u)
                if u % 2 == 1:
                    _tail_quad(u // 2, h)
                    g = u // 2
                    # stream outputs to keep the tail short (chunk k covers
                    # quads 16k..16k+15, all complete by g = 16k+16+9)
                    if g % 16 == 9 and g > 16:
                        k = g // 16 - 1
                        nc.sync.dma_start(out=mv_d[:, 32 * k:32 * (k + 1)],
                                          in_=mv_sb[:, 32 * k:32 * (k + 1)])
                        nc.sync.dma_start(out=ql_d[:, 16 * k:16 * (k + 1)],
                                          in_=ql_sb[:, 16 * k:16 * (k + 1)])
                        nc.sync.dma_start(out=cr_d[:, 16 * k:16 * (k + 1)],
                                          in_=cr_sb[:, 16 * k:16 * (k + 1)])

            k = 3
            nc.sync.dma_start(out=mv_d[:, 32 * k:32 * (k + 1)],
                              in_=mv_sb[:, 32 * k:32 * (k + 1)])
            nc.sync.dma_start(out=ql_d[:, 16 * k:16 * (k + 1)],
                              in_=ql_sb[:, 16 * k:16 * (k + 1)])
            nc.sync.dma_start(out=cr_d[:, 16 * k:16 * (k + 1)],
                              in_=cr_sb[:, 16 * k:16 * (k + 1)])

    nc.finalize()
    return nc


_NC_CACHE = {}


def _get_program():
    if "nc" not in _NC_CACHE:
        _NC_CACHE["nc"] = _build_program()
    return _NC_CACHE["nc"]


def _make_consts(W1, b1, W2):
    w1full = np.zeros((128, HID), dtype=NP_BF16)
    for j in range(4):
        w1full[32 * j:32 * j + D, :] = W1.astype(NP_BF16)
    return {
        "w1full": w1full,
        "w2": np.ascontiguousarray(W2.astype(NP_BF16)),
        "b1col": np.ascontiguousarray(b1.astype(np.float32).reshape(HID, 1)),
    }


def kernel(states, actions, rewards, W1, b1, W2, b2, _run_kwargs=None):
    states = np.asarray(states, dtype=np.float32)
    actions = np.asarray(actions, dtype=np.float32)
    rewards = np.asarray(rewards, dtype=np.float32)
    W1 = np.asarray(W1, dtype=np.float32)
    b1 = np.asarray(b1, dtype=np.float32)
    W2 = np.asarray(W2, dtype=np.float32)
    b2 = np.asarray(b2, dtype=np.float32)

    consts = _make_consts(W1, b1, W2)

    # prepack per-core device layouts:
    #   st_pre[16j+dd, g*T+t] = states[core*SS + 4g+j, dd, t]   (bf16)
    #   at_pre[4j+d,  g*T+t] = b2[d] - actions[core*SS + 4g+j, d, t]  (bf16)
    st_all = states.reshape(N_CORES, SS // 4, 4, D, T)
    st_all = np.ascontiguousarray(st_all.transpose(0, 2, 3, 1, 4)).astype(NP_BF16)
    st_all = st_all.reshape(N_CORES, 64, NQ * T)
    aadj = b2[None, :, None] - actions
    at_all = aadj.reshape(N_CORES, SS // 4, 4, A, T)
    at_all = np.ascontiguousarray(at_all.transpose(0, 2, 3, 1, 4)).astype(NP_BF16)
    at_all = at_all.reshape(N_CORES, 16, NQ * T)

    in_maps = []
    for c in range(N_CORES):
        m = {"st_pre": st_all[c], "at_pre": at_all[c]}
        m.update(consts)
        in_maps.append(m)

    nc = _get_program()
    res = run_bass_kernel_spmd(nc, in_maps, core_ids=list(range(N_CORES)),
                               **(_run_kwargs or {}))
    results = res.results

    # host combine in float64
    C0 = -0.5 * A * np.log(2.0 * np.pi * SD_VAR)
    mx_pos = np.log(1.0 / (2.0 * MAX_POSITION))
    rew = rewards.astype(np.float64)
    R_all = rew.sum(axis=1)            # [S]
    rlast_all = rew[:, -1]             # [S]
    total = 0.0
    for c in range(N_CORES):
        mv = results[c]["mv"].astype(np.float64)      # [128, 2*NQ]
        qlv = results[c]["ql"].astype(np.float64)     # [128, NQ] = v at T-1
        crv = results[c]["cr"].astype(np.float64)     # [128, NQ] = sum v*c
        mean = mv[:, 0::2]
        var = mv[:, 1::2]
        sum_v2 = T * (var + mean * mean)              # [128, NQ]
        # partition p = 32j + d (d < A), sim s_local = 4g + j
        at64 = at_all[c].astype(np.float64).reshape(4, A, NQ, T)  # [j,d,g,t]
        c2 = (at64 ** 2).sum(axis=(1, 3))             # [j, g] = sum_{d,t} c^2
        clast = at64[:, :, :, -1]                     # [j, d, g]
        sel_v2 = sum_v2.reshape(4, 32, NQ)[:, :A, :]  # [j, d, g]
        sel_cr = crv.reshape(4, 32, NQ)[:, :A, :]
        sel_ql = qlv.reshape(4, 32, NQ)[:, :A, :]
        q_sum = (sel_v2 + 2.0 * sel_cr).sum(axis=1) + c2          # [j, g]
        q_sum = q_sum.T.reshape(SS)                   # s_local = 4g + j
        q_last = ((sel_ql + clast) ** 2).sum(axis=1).T.reshape(SS)
        sl = slice(SS * c, SS * (c + 1))
        L = -0.5 * q_sum / SD_VAR + T * C0
        ll_last = -0.5 * q_last / SD_VAR + C0
        A_sum = (R_all[sl] + rlast_all[sl]
                 - ALPHA * (L + ll_last) - T * mx_pos)
        total += np.sum(A_sum * L)
    out = np.float32(total / S)
    if _run_kwargs:
        _NC_CACHE["last_result"] = res
    return out


if __name__ == "__main__":
    rng = np.random.default_rng(0)
    inputs = {
        "states": rng.standard_normal((S, D, T), dtype=np.float32),
        "actions": rng.standard_normal((S, A, T), dtype=np.float32),
        "rewards": rng.standard_normal((S, T), dtype=np.float32),
        "W1": (rng.standard_normal((D, HID)) / np.sqrt(D)).astype(np.float32),
        "b1": np.zeros(HID, np.float32),
        "W2": (rng.standard_normal((HID, A)) / np.sqrt(HID)).astype(np.float32),
        "b2": np.zeros(A, np.float32),
    }
    print("result:", kernel(**inputs))


# revision 20
# speedup vs baseline: 1.0502x; 1.0502x over previous
"""Trainium2 Bass kernel for nn_MEPG_Loss (MEPG policy-gradient loss).

Math (forward only; stop_gradient is identity):
    h   = tanh(states[s,:,t] @ W1 + b1)                  [S,T,H]
    mu  = h @ W2 + b2                                    [S,T,A]
    ll[s,t] = -0.5*(||a[s,:,t]-mu||^2/SD + A*log(2*pi*SD))
    out = sum_s (sum_t A_hat[t,s]) * (sum_t ll[t,s]) / S

Per-simulation reductions with v = W2^T h (device) and c = b2 - a (host):
    q_sum[s] = sum_t ||v+c||^2 = sum_t ||v||^2 + 2 sum_t <v,c> + sum_t ||c||^2
      - sum_t v, sum_t v^2 per partition: ONE bn_stats on the mu psum bank
      - cross = sum_t v*c per partition: ONE scalar_tensor_tensor accum
      - sum_t ||c||^2, rewards sums: host numpy (inputs are host-resident)
    q_last[s]: copy v[:, T-1], combine with host c[:, T-1]

Device pipeline, per core (256 sims as 64 quads of 4 sims):
    - states prepacked on host to [64, NQ*T] bf16; contiguous block DMAs
      (HWDGE via sync queue; gpsimd SWDGE costs ~600ns/DMA of Q7 time)
    - mm1: 4 row-tiled K=16 matmuls -> 2-bank psum units (3-unit rotation;
      the ACT is each unit's ONLY reader so mm1 runs 1.5 quads ahead)
    - ScalarE: one merged tanh over 2048 cols when the quad's two units are
      adjacent (2/3 of quads), else two 1024-col tanhs.  ScalarE is the
      bottleneck engine: ~1 elem/lane/cycle @ 1.2 GHz, ~127 us total.
    - mm2: 4 col-tiled matmuls (lhsT=W2) -> mu psum bank (x2 rotation)
    - DVE: stt cross-term + qlast copy + bn_stats + bn_aggr per quad
Final combine (tiny) in float64 on host.
"""

import os
import sys

import numpy as np

if not any(os.path.isdir(os.path.join(p, "concourse")) for p in sys.path if p):
    sys.path.insert(0, "/opt/trn_rl_repo")

import ml_dtypes

import concourse.bacc as bacc
import concourse.tile as tile
from concourse import mybir
from concourse.bass_utils import run_bass_kernel_spmd

# Problem constants (hardcoded per contract)
S, D, A, T, HID = 2048, 16, 4, 512, 128
N_CORES = 8
SS = S // N_CORES          # 256 sims per core
NQ = SS // 4               # 64 quads per core
SD_VAR = 0.04
ALPHA = 0.1
MAX_POSITION = 1.0

# DMA blocks of quads: small first blocks so the first mm1/tanh start early
BLOCK_SIZES = [1, 1, 2] + [4] * 15
BLOCK_Q0 = np.concatenate([[0], np.cumsum(BLOCK_SIZES)[:-1]]).tolist()
NBL = len(BLOCK_SIZES)
QUAD_BLOCK = []
for bi, (q0, nq) in enumerate(zip(BLOCK_Q0, BLOCK_SIZES)):
    QUAD_BLOCK += [bi] * nq

F32 = mybir.dt.float32
BF16 = mybir.dt.bfloat16
NP_BF16 = ml_dtypes.bfloat16


def _build_program():
    nc = bacc.Bacc("TRN2", target_bir_lowering=False, debug=False)

    stp_d = nc.dram_tensor("st_pre", [64, NQ * T], BF16, kind="ExternalInput").ap()
    atp_d = nc.dram_tensor("at_pre", [16, NQ * T], BF16, kind="ExternalInput").ap()
    w1f_d = nc.dram_tensor("w1full", [128, HID], BF16, kind="ExternalInput").ap()
    w2_d = nc.dram_tensor("w2", [HID, A], BF16, kind="ExternalInput").ap()
    b1_d = nc.dram_tensor("b1col", [HID, 1], F32, kind="ExternalInput").ap()

    mv_d = nc.dram_tensor("mv", [128, 2 * NQ], F32, kind="ExternalOutput").ap()
    ql_d = nc.dram_tensor("ql", [128, NQ], F32, kind="ExternalOutput").ap()
    cr_d = nc.dram_tensor("cr", [128, NQ], F32, kind="ExternalOutput").ap()

    with tile.TileContext(nc) as tc:
        with (
            tc.tile_pool(name="consts", bufs=1) as consts,
            tc.tile_pool(name="stp", bufs=4) as stp,
            tc.tile_pool(name="atp", bufs=4) as atp,
            tc.tile_pool(name="hsb", bufs=2) as hsb,
            tc.tile_pool(name="sdp", bufs=2) as sdp,
            tc.tile_pool(name="bstp", bufs=2) as bstp,
            tc.tile_pool(name="outs", bufs=1) as outp,
            tc.tile_pool(name="hpp", bufs=1, space="PSUM") as hpp,
        ):
            # One persistent PSUM tile covering all 8 banks, hand-carved:
            # 2 h_pre buffers of 3 banks (3 sims each) + 2 mu banks.
            # Each tanh reads exactly one buffer, which is freed right when
            # the next tanh starts -> mm1 always runs one unit ahead with a
            # full ACT window of slack, no unit sharing between quads.
            PS = hpp.tile([128, 4096], F32, tag="PS")
            BUF = [PS[:, 0:1536], PS[:, 1536:3072]]
            MU = [PS[:, 3072:3584], PS[:, 3584:4096]]

            # w1 + the first data block go on the DMA queues before anything
            # else so mm1(0) can start ASAP
            w1t = consts.tile([128, HID], BF16, tag="w1t")
            w2t = consts.tile([HID, A], BF16, tag="w2t")
            b1t = consts.tile([HID, 1], F32, tag="b1t")
            nc.sync.dma_start(out=w1t[:], in_=w1f_d)

            mv_sb = outp.tile([128, 2 * NQ], F32, tag="mv")
            ql_sb = outp.tile([128, NQ], F32, tag="ql")
            cr_sb = outp.tile([128, NQ], F32, tag="cr")

            blocks = {}

            def _ensure_block(bi):
                if bi in blocks or bi >= NBL:
                    return
                q0, nq = BLOCK_Q0[bi], BLOCK_SIZES[bi]
                c0 = T * q0
                st = stp.tile([128, nq * T], BF16, tag=f"st{nq}",
                              name=f"st_{bi}")
                at = atp.tile([128, nq * T], BF16, tag=f"at{nq}",
                              name=f"at_{bi}")
                for j in range(4):
                    nc.sync.dma_start(
                        out=st[32 * j:32 * j + D, :],
                        in_=stp_d[D * j:D * (j + 1), c0:c0 + nq * T],
                    )
                for j in range(4):
                    nc.sync.dma_start(
                        out=at[32 * j:32 * j + A, :],
                        in_=atp_d[A * j:A * (j + 1), c0:c0 + nq * T],
                    )
                blocks[bi] = (st, at)

            def _nsim(B):
                return min(3, SS - 3 * B)

            def _mm1u(B):
                # unit B = sims 3B .. 3B+ns-1, one 3-MM burst into BUF[B%2]
                buf = BUF[B % 2]
                for i in range(_nsim(B)):
                    s = 3 * B + i
                    g, j = s // 4, s % 4
                    bi = QUAD_BLOCK[g]
                    st, _ = blocks[bi]
                    q = g - BLOCK_Q0[bi]
                    nc.tensor.matmul(
                        out=buf[:, T * i:T * (i + 1)],
                        lhsT=w1t[32 * j:32 * j + D, :],
                        rhs=st[32 * j:32 * j + D, T * q:T * (q + 1)],
                        start=True, stop=True,
                        tile_position=(32 * j, 0),
                    )

            hmap = {}

            def _act_u(B):
                ns = _nsim(B)
                h = hsb.tile([128, 3 * T], BF16, tag="h", name=f"h_{B}")
                nc.scalar.activation(
                    out=h[:, 0:ns * T], in_=BUF[B % 2][:, 0:ns * T],
                    func=mybir.ActivationFunctionType.Tanh,
                    bias=b1t[:], scale=1.0,
                )
                hmap[B] = h

            def _tail_quad(g):
                bi = QUAD_BLOCK[g]
                _, at = blocks[bi]
                q = g - BLOCK_Q0[bi]
                mu = MU[g % 2]
                for j in range(4):
                    s = 4 * g + j
                    h = hmap[s // 3]
                    nc.tensor.matmul(
                        out=mu[32 * j:32 * j + A, :],
                        lhsT=w2t[:],
                        rhs=h[:, T * (s % 3):T * (s % 3 + 1)],
                        start=True, stop=True,
                        tile_position=(0, 32 * j),
                        skip_group_check=True,
                    )
                # cross = sum_t v*c per partition (c = b2-a, bf16)
                sd = sdp.tile([128, T], BF16, tag="sd", name=f"sd_{g}")
                nc.vector.scalar_tensor_tensor(
                    out=sd[:], in0=mu[:], scalar=1.0,
                    in1=at[:, T * q:T * (q + 1)],
                    op0=mybir.AluOpType.mult, op1=mybir.AluOpType.mult,
                    accum_out=cr_sb[:, g:g + 1],
                )
                nc.vector.tensor_copy(ql_sb[:, g:g + 1], mu[:, T - 1:T])
                sts = bstp.tile([128, 6], F32, tag="bst", name=f"bst_{g}")
                nc.vector.bn_stats(out=sts[:], in_=mu[:])
                nc.vector.bn_aggr(out=mv_sb[:, 2 * g:2 * g + 2], in_=sts[:])

            _ensure_block(0)
            _ensure_block(1)
            # dummy activation: forces the tanh table load at t~0
            dums = consts.tile([128, 1], F32, tag="dums")
            dumo = consts.tile([128, 1], F32, tag="dumo")
            nc.vector.memset(dums[:], 0.0)
            nc.scalar.activation(
                out=dumo[:], in_=dums[:],
                func=mybir.ActivationFunctionType.Tanh, scale=1.0,
            )
            nc.sync.dma_start(out=w2t[:], in_=w2_d)
            nc.sync.dma_start(out=b1t[:], in_=b1_d)
            _ensure_block(2)
            _ensure_block(3)

            # uniform 3-sim-unit pipeline: mm1(B+1) fills the other buffer
            # during ACT(B); quad tails run right after the ACT that
            # completes their 4th sim
            NU = (SS + 2) // 3
            _mm1u(0)
            for B in range(NU):
                if B + 1 < NU:
                    glast = min(NQ - 1, (3 * (B + 1) + 2) // 4)
                    for x in range(QUAD_BLOCK[glast] + 3):
                        _ensure_block(x)
                    _mm1u(B + 1)
                _act_u(B)
                for g in range(NQ):
                    if (4 * g + 3) // 3 == B:
                        _tail_quad(g)
                        # stream outputs (chunk k = quads 16k..16k+15)
                        if g % 16 == 9 and g > 16:
                            k = g // 16 - 1
                            nc.sync.dma_start(
                                out=mv_d[:, 32 * k:32 * (k + 1)],
                                in_=mv_sb[:, 32 * k:32 * (k + 1)])
                            nc.sync.dma_start(
                                out=ql_d[:, 16 * k:16 * (k + 1)],
                                in_=ql_sb[:, 16 * k:16 * (k + 1)])
                            nc.sync.dma_start(
                                out=cr_d[:, 16 * k:16 * (k + 1)],
                                in_=cr_sb[:, 16 * k:16 * (k + 1)])

            k = 3
            nc.sync.dma_start(out=mv_d[:, 32 * k:32 * (k + 1)],
                              in_=mv_sb[:, 32 * k:32 * (k + 1)])
            nc.sync.dma_start(out=ql_d[:, 16 * k:16 * (k + 1)],
                              in_=ql_sb[:, 16 * k:16 * (k + 1)])
            nc.sync.dma_start(out=cr_d[:, 16 * k:16 * (k + 1)],
                              in_=cr_sb[:, 16 * k:16 * (k + 1)])

    nc.finalize()
    return nc


_NC_CACHE = {}


def _get_program():
    if "nc" not in _NC_CACHE:
        _NC_CACHE["nc"] = _build_program()
    return _NC_CACHE["nc"]


def _make_consts(W1, b1, W2):
    w1full = np.zeros((128, HID), dtype=NP_BF16)
    for j in range(4):
        w1full[32 * j:32 * j + D, :] = W1.astype(NP_BF16)
    return {
        "w1full": w1full,
        "w2": np.ascontiguousarray(W2.astype(NP_BF16)),
        "b1col": np.ascontiguousarray(b1.astype(np.float32).reshape(HID, 1)),
    }


def kernel(states, actions, rewards, W1, b1, W2, b2, _run_kwargs=None):
    states = np.asarray(states, dtype=np.float32)
    actions = np.asarray(actions, dtype=np.float32)
    rewards = np.asarray(rewards, dtype=np.float32)
    W1 = np.asarray(W1, dtype=np.float32)
    b1 = np.asarray(b1, dtype=np.float32)
    W2 = np.asarray(W2, dtype=np.float32)
    b2 = np.asarray(b2, dtype=np.float32)

    consts = _make_consts(W1, b1, W2)

    # prepack per-core device layouts:
    #   st_pre[16j+dd, g*T+t] = states[core*SS + 4g+j, dd, t]   (bf16)
    #   at_pre[4j+d,  g*T+t] = b2[d] - actions[core*SS + 4g+j, d, t]  (bf16)
    st_all = states.reshape(N_CORES, SS // 4, 4, D, T)
    st_all = np.ascontiguousarray(st_all.transpose(0, 2, 3, 1, 4)).astype(NP_BF16)
    st_all = st_all.reshape(N_CORES, 64, NQ * T)
    aadj = b2[None, :, None] - actions
    at_all = aadj.reshape(N_CORES, SS // 4, 4, A, T)
    at_all = np.ascontiguousarray(at_all.transpose(0, 2, 3, 1, 4)).astype(NP_BF16)
    at_all = at_all.reshape(N_CORES, 16, NQ * T)

    in_maps = []
    for c in range(N_CORES):
        m = {"st_pre": st_all[c], "at_pre": at_all[c]}
        m.update(consts)
        in_maps.append(m)

    nc = _get_program()
    res = run_bass_kernel_spmd(nc, in_maps, core_ids=list(range(N_CORES)),
                               **(_run_kwargs or {}))
    results = res.results

    # host combine in float64
    C0 = -0.5 * A * np.log(2.0 * np.pi * SD_VAR)
    mx_pos = np.log(1.0 / (2.0 * MAX_POSITION))
    rew = rewards.astype(np.float64)
    R_all = rew.sum(axis=1)            # [S]
    rlast_all = rew[:, -1]             # [S]
    total = 0.0
    for c in range(N_CORES):
        mv = results[c]["mv"].astype(np.float64)      # [128, 2*NQ]
        qlv = results[c]["ql"].astype(np.float64)     # [128, NQ] = v at T-1
        crv = results[c]["cr"].astype(np.float64)     # [128, NQ] = sum v*c
        mean = mv[:, 0::2]
        var = mv[:, 1::2]
        sum_v2 = T * (var + mean * mean)              # [128, NQ]
        # partition p = 32j + d (d < A), sim s_local = 4g + j
        at64 = at_all[c].astype(np.float64).reshape(4, A, NQ, T)  # [j,d,g,t]
        c2 = (at64 ** 2).sum(axis=(1, 3))             # [j, g] = sum_{d,t} c^2
        clast = at64[:, :, :, -1]                     # [j, d, g]
        sel_v2 = sum_v2.reshape(4, 32, NQ)[:, :A, :]  # [j, d, g]
        sel_cr = crv.reshape(4, 32, NQ)[:, :A, :]
        sel_ql = qlv.reshape(4, 32, NQ)[:, :A, :]
        q_sum = (sel_v2 + 2.0 * sel_cr).sum(axis=1) + c2          # [j, g]
        q_sum = q_sum.T.reshape(SS)                   # s_local = 4g + j
        q_last = ((sel_ql + clast) ** 2).sum(axis=1).T.reshape(SS)
        sl = slice(SS * c, SS * (c + 1))
        L = -0.5 * q_sum / SD_VAR + T * C0
        ll_last = -0.5 * q_last / SD_VAR + C0
        A_sum = (R_all[sl] + rlast_all[sl]
                 - ALPHA * (L + ll_last) - T * mx_pos)
        total += np.sum(A_sum * L)
    out = np.float32(total / S)
    if _run_kwargs:
        _NC_CACHE["last_result"] = res
    return out


if __name__ == "__main__":
    rng = np.random.default_rng(0)
    inputs = {
        "states": rng.standard_normal((S, D, T), dtype=np.float32),
        "actions": rng.standard_normal((S, A, T), dtype=np.float32),
        "rewards": rng.standard_normal((S, T), dtype=np.float32),
        "W1": (rng.standard_normal((D, HID)) / np.sqrt(D)).astype(np.float32),
        "b1": np.zeros(HID, np.float32),
        "W2": (rng.standard_normal((HID, A)) / np.sqrt(HID)).astype(np.float32),
        "b2": np.zeros(A, np.float32),
    }
    print("result:", kernel(**inputs))


# revision 21
# speedup vs baseline: 1.0832x; 1.0314x over previous
"""Trainium2 Bass kernel for nn_MEPG_Loss (MEPG policy-gradient loss).

Math (forward only; stop_gradient is identity):
    h   = tanh(states[s,:,t] @ W1 + b1)                  [S,T,H]
    mu  = h @ W2 + b2                                    [S,T,A]
    ll[s,t] = -0.5*(||a[s,:,t]-mu||^2/SD + A*log(2*pi*SD))
    out = sum_s (sum_t A_hat[t,s]) * (sum_t ll[t,s]) / S

Per-simulation reductions with v = W2^T h (device) and c = b2 - a (host):
    q_sum[s] = sum_t ||v+c||^2 = sum_t ||v||^2 + 2 sum_t <v,c> + sum_t ||c||^2
      - sum_t v, sum_t v^2 per partition: ONE bn_stats on the mu psum bank
      - cross = sum_t v*c per partition: ONE scalar_tensor_tensor accum
      - sum_t ||c||^2, rewards sums: host numpy (inputs are host-resident)
    q_last[s]: copy v[:, T-1], combine with host c[:, T-1]

Device pipeline, per core (256 sims as 64 quads of 4 sims):
    - states prepacked on host to [64, NQ*T] bf16; contiguous block DMAs
      (HWDGE via sync queue; gpsimd SWDGE costs ~600ns/DMA of Q7 time)
    - mm1: 4 row-tiled K=16 matmuls -> 2-bank psum units (3-unit rotation;
      the ACT is each unit's ONLY reader so mm1 runs 1.5 quads ahead)
    - ScalarE: one merged tanh over 2048 cols when the quad's two units are
      adjacent (2/3 of quads), else two 1024-col tanhs.  ScalarE is the
      bottleneck engine: ~1 elem/lane/cycle @ 1.2 GHz, ~127 us total.
    - mm2: 4 col-tiled matmuls (lhsT=W2) -> mu psum bank (x2 rotation)
    - DVE: stt cross-term + qlast copy + bn_stats + bn_aggr per quad
Final combine (tiny) in float64 on host.
"""

import os
import sys

import numpy as np

if not any(os.path.isdir(os.path.join(p, "concourse")) for p in sys.path if p):
    sys.path.insert(0, "/opt/trn_rl_repo")

import ml_dtypes

import concourse.bacc as bacc
import concourse.tile as tile
from concourse import mybir
from concourse.bass_utils import run_bass_kernel_spmd

# Problem constants (hardcoded per contract)
S, D, A, T, HID = 2048, 16, 4, 512, 128
N_CORES = 8
SS = S // N_CORES          # 256 sims per core
NQ = SS // 4               # 64 quads per core
SD_VAR = 0.04
ALPHA = 0.1
MAX_POSITION = 1.0

# DMA blocks of quads: small first blocks so the first mm1/tanh start early
BLOCK_SIZES = [1, 1, 2] + [4] * 15
BLOCK_Q0 = np.concatenate([[0], np.cumsum(BLOCK_SIZES)[:-1]]).tolist()
NBL = len(BLOCK_SIZES)
QUAD_BLOCK = []
for bi, (q0, nq) in enumerate(zip(BLOCK_Q0, BLOCK_SIZES)):
    QUAD_BLOCK += [bi] * nq

F32 = mybir.dt.float32
BF16 = mybir.dt.bfloat16
NP_BF16 = ml_dtypes.bfloat16


def _build_program():
    nc = bacc.Bacc("TRN2", target_bir_lowering=False, debug=False)

    stp_d = nc.dram_tensor("st_pre", [64, NQ * T], BF16, kind="ExternalInput").ap()
    atp_d = nc.dram_tensor("at_pre", [16, NQ * T], BF16, kind="ExternalInput").ap()
    w1f_d = nc.dram_tensor("w1full", [128, HID], BF16, kind="ExternalInput").ap()
    w2_d = nc.dram_tensor("w2", [HID, A], BF16, kind="ExternalInput").ap()
    b1_d = nc.dram_tensor("b1col", [HID, 1], F32, kind="ExternalInput").ap()

    st6_d = nc.dram_tensor("st6", [128, 6 * NQ], F32, kind="ExternalOutput").ap()
    ql_d = nc.dram_tensor("ql", [128, NQ], F32, kind="ExternalOutput").ap()
    cr_d = nc.dram_tensor("cr", [128, NQ], F32, kind="ExternalOutput").ap()

    with tile.TileContext(nc) as tc:
        with (
            tc.tile_pool(name="consts", bufs=1) as consts,
            tc.tile_pool(name="stp", bufs=4) as stp,
            tc.tile_pool(name="atp", bufs=4) as atp,
            tc.tile_pool(name="hsb", bufs=2) as hsb,
            tc.tile_pool(name="sdp", bufs=2) as sdp,
            tc.tile_pool(name="outs", bufs=1) as outp,
            tc.tile_pool(name="hpp", bufs=1, space="PSUM") as hpp,
        ):
            # One persistent PSUM tile covering all 8 banks, hand-carved:
            # 2 h_pre buffers of 3 banks (3 sims each) + 2 mu banks.
            # Each tanh reads exactly one buffer, which is freed right when
            # the next tanh starts -> mm1 always runs one unit ahead with a
            # full ACT window of slack, no unit sharing between quads.
            PS = hpp.tile([128, 4096], F32, tag="PS")
            BUF = [PS[:, 0:1536], PS[:, 1536:3072]]
            MU = [PS[:, 3072:3584], PS[:, 3584:4096]]

            # w1 + the first data block go on the DMA queues before anything
            # else so mm1(0) can start ASAP
            w1t = consts.tile([128, HID], BF16, tag="w1t")
            w2t = consts.tile([HID, A], BF16, tag="w2t")
            b1t = consts.tile([HID, 1], F32, tag="b1t")
            nc.sync.dma_start(out=w1t[:], in_=w1f_d)

            st6_sb = outp.tile([128, 6 * NQ], F32, tag="st6")
            ql_sb = outp.tile([128, NQ], F32, tag="ql")
            cr_sb = outp.tile([128, NQ], F32, tag="cr")

            blocks = {}

            def _ensure_block(bi):
                if bi in blocks or bi >= NBL:
                    return
                q0, nq = BLOCK_Q0[bi], BLOCK_SIZES[bi]
                c0 = T * q0
                st = stp.tile([128, nq * T], BF16, tag=f"st{nq}",
                              name=f"st_{bi}")
                at = atp.tile([128, nq * T], BF16, tag=f"at{nq}",
                              name=f"at_{bi}")
                for j in range(4):
                    nc.sync.dma_start(
                        out=st[32 * j:32 * j + D, :],
                        in_=stp_d[D * j:D * (j + 1), c0:c0 + nq * T],
                    )
                for j in range(4):
                    nc.sync.dma_start(
                        out=at[32 * j:32 * j + A, :],
                        in_=atp_d[A * j:A * (j + 1), c0:c0 + nq * T],
                    )
                blocks[bi] = (st, at)

            def _nsim(B):
                return min(3, SS - 3 * B)

            def _mm1u(B):
                # unit B = sims 3B .. 3B+ns-1, one 3-MM burst into BUF[B%2]
                buf = BUF[B % 2]
                for i in range(_nsim(B)):
                    s = 3 * B + i
                    g, j = s // 4, s % 4
                    bi = QUAD_BLOCK[g]
                    st, _ = blocks[bi]
                    q = g - BLOCK_Q0[bi]
                    nc.tensor.matmul(
                        out=buf[:, T * i:T * (i + 1)],
                        lhsT=w1t[32 * j:32 * j + D, :],
                        rhs=st[32 * j:32 * j + D, T * q:T * (q + 1)],
                        start=True, stop=True,
                        tile_position=(32 * j, 0),
                    )

            hmap = {}

            def _act_u(B):
                ns = _nsim(B)
                h = hsb.tile([128, 3 * T], BF16, tag="h", name=f"h_{B}")
                nc.scalar.activation(
                    out=h[:, 0:ns * T], in_=BUF[B % 2][:, 0:ns * T],
                    func=mybir.ActivationFunctionType.Tanh,
                    bias=b1t[:], scale=1.0,
                )
                hmap[B] = h

            def _tail_quad(g):
                bi = QUAD_BLOCK[g]
                _, at = blocks[bi]
                q = g - BLOCK_Q0[bi]
                mu = MU[g % 2]
                for j in range(4):
                    s = 4 * g + j
                    h = hmap[s // 3]
                    nc.tensor.matmul(
                        out=mu[32 * j:32 * j + A, :],
                        lhsT=w2t[:],
                        rhs=h[:, T * (s % 3):T * (s % 3 + 1)],
                        start=True, stop=True,
                        tile_position=(0, 32 * j),
                        skip_group_check=True,
                    )
                # cross = sum_t v*c per partition (c = b2-a, bf16)
                sd = sdp.tile([128, T], BF16, tag="sd", name=f"sd_{g}")
                nc.vector.scalar_tensor_tensor(
                    out=sd[:], in0=mu[:], scalar=1.0,
                    in1=at[:, T * q:T * (q + 1)],
                    op0=mybir.AluOpType.mult, op1=mybir.AluOpType.mult,
                    accum_out=cr_sb[:, g:g + 1],
                )
                if g % 2 == 1:
                    # v[:, T-1] for both mu banks (cols 3583 and 4095 of PS)
                    nc.vector.tensor_copy(ql_sb[:, g - 1:g + 1],
                                          PS[:, 3583:4096:512])
                nc.vector.bn_stats(out=st6_sb[:, 6 * g:6 * (g + 1)],
                                   in_=mu[:])

            _ensure_block(0)
            _ensure_block(1)
            # dummy activation: forces the tanh table load at t~0
            dums = consts.tile([128, 1], F32, tag="dums")
            dumo = consts.tile([128, 1], F32, tag="dumo")
            nc.vector.memset(dums[:], 0.0)
            nc.scalar.activation(
                out=dumo[:], in_=dums[:],
                func=mybir.ActivationFunctionType.Tanh, scale=1.0,
            )
            nc.sync.dma_start(out=w2t[:], in_=w2_d)
            nc.sync.dma_start(out=b1t[:], in_=b1_d)
            _ensure_block(2)
            _ensure_block(3)

            # uniform 3-sim-unit pipeline: mm1(B+1) fills the other buffer
            # during ACT(B); quad tails run right after the ACT that
            # completes their 4th sim
            NU = (SS + 2) // 3
            _mm1u(0)
            for B in range(NU):
                if B + 1 < NU:
                    glast = min(NQ - 1, (3 * (B + 1) + 2) // 4)
                    for x in range(QUAD_BLOCK[glast] + 3):
                        _ensure_block(x)
                    _mm1u(B + 1)
                _act_u(B)
                for g in range(NQ):
                    if (4 * g + 3) // 3 == B:
                        _tail_quad(g)
                        # stream outputs (chunk k = quads 16k..16k+15)
                        if g % 16 == 9 and g > 16:
                            k = g // 16 - 1
                            nc.sync.dma_start(
                                out=st6_d[:, 96 * k:96 * (k + 1)],
                                in_=st6_sb[:, 96 * k:96 * (k + 1)])
                            nc.sync.dma_start(
                                out=ql_d[:, 16 * k:16 * (k + 1)],
                                in_=ql_sb[:, 16 * k:16 * (k + 1)])
                            nc.sync.dma_start(
                                out=cr_d[:, 16 * k:16 * (k + 1)],
                                in_=cr_sb[:, 16 * k:16 * (k + 1)])

            k = 3
            nc.sync.dma_start(out=st6_d[:, 96 * k:96 * (k + 1)],
                              in_=st6_sb[:, 96 * k:96 * (k + 1)])
            nc.sync.dma_start(out=ql_d[:, 16 * k:16 * (k + 1)],
                              in_=ql_sb[:, 16 * k:16 * (k + 1)])
            nc.sync.dma_start(out=cr_d[:, 16 * k:16 * (k + 1)],
                              in_=cr_sb[:, 16 * k:16 * (k + 1)])

    nc.finalize()
    return nc


_NC_CACHE = {}


def _get_program():
    if "nc" not in _NC_CACHE:
        _NC_CACHE["nc"] = _build_program()
    return _NC_CACHE["nc"]


def _make_consts(W1, b1, W2):
    w1full = np.zeros((128, HID), dtype=NP_BF16)
    for j in range(4):
        w1full[32 * j:32 * j + D, :] = W1.astype(NP_BF16)
    return {
        "w1full": w1full,
        "w2": np.ascontiguousarray(W2.astype(NP_BF16)),
        "b1col": np.ascontiguousarray(b1.astype(np.float32).reshape(HID, 1)),
    }


def kernel(states, actions, rewards, W1, b1, W2, b2, _run_kwargs=None):
    states = np.asarray(states, dtype=np.float32)
    actions = np.asarray(actions, dtype=np.float32)
    rewards = np.asarray(rewards, dtype=np.float32)
    W1 = np.asarray(W1, dtype=np.float32)
    b1 = np.asarray(b1, dtype=np.float32)
    W2 = np.asarray(W2, dtype=np.float32)
    b2 = np.asarray(b2, dtype=np.float32)

    consts = _make_consts(W1, b1, W2)

    # prepack per-core device layouts:
    #   st_pre[16j+dd, g*T+t] = states[core*SS + 4g+j, dd, t]   (bf16)
    #   at_pre[4j+d,  g*T+t] = b2[d] - actions[core*SS + 4g+j, d, t]  (bf16)
    st_all = states.reshape(N_CORES, SS // 4, 4, D, T)
    st_all = np.ascontiguousarray(st_all.transpose(0, 2, 3, 1, 4)).astype(NP_BF16)
    st_all = st_all.reshape(N_CORES, 64, NQ * T)
    aadj = b2[None, :, None] - actions
    at_all = aadj.reshape(N_CORES, SS // 4, 4, A, T)
    at_all = np.ascontiguousarray(at_all.transpose(0, 2, 3, 1, 4)).astype(NP_BF16)
    at_all = at_all.reshape(N_CORES, 16, NQ * T)

    in_maps = []
    for c in range(N_CORES):
        m = {"st_pre": st_all[c], "at_pre": at_all[c]}
        m.update(consts)
        in_maps.append(m)

    nc = _get_program()
    res = run_bass_kernel_spmd(nc, in_maps, core_ids=list(range(N_CORES)),
                               **(_run_kwargs or {}))
    results = res.results

    # host combine in float64
    C0 = -0.5 * A * np.log(2.0 * np.pi * SD_VAR)
    mx_pos = np.log(1.0 / (2.0 * MAX_POSITION))
    rew = rewards.astype(np.float64)
    R_all = rew.sum(axis=1)            # [S]
    rlast_all = rew[:, -1]             # [S]
    total = 0.0
    for c in range(N_CORES):
        st6 = results[c]["st6"].astype(np.float64).reshape(128, NQ, 6)
        qlv = results[c]["ql"].astype(np.float64)     # [128, NQ] = v at T-1
        crv = results[c]["cr"].astype(np.float64)     # [128, NQ] = sum v*c
        # bn_stats raw: [n0, mean0, M2_0, n1, mean1, M2_1] per partition
        sum_v2 = (st6[:, :, 2] + st6[:, :, 0] * st6[:, :, 1] ** 2
                  + st6[:, :, 5] + st6[:, :, 3] * st6[:, :, 4] ** 2)
        # partition p = 32j + d (d < A), sim s_local = 4g + j
        at64 = at_all[c].astype(np.float64).reshape(4, A, NQ, T)  # [j,d,g,t]
        c2 = (at64 ** 2).sum(axis=(1, 3))             # [j, g] = sum_{d,t} c^2
        clast = at64[:, :, :, -1]                     # [j, d, g]
        sel_v2 = sum_v2.reshape(4, 32, NQ)[:, :A, :]  # [j, d, g]
        sel_cr = crv.reshape(4, 32, NQ)[:, :A, :]
        sel_ql = qlv.reshape(4, 32, NQ)[:, :A, :]
        q_sum = (sel_v2 + 2.0 * sel_cr).sum(axis=1) + c2          # [j, g]
        q_sum = q_sum.T.reshape(SS)                   # s_local = 4g + j
        q_last = ((sel_ql + clast) ** 2).sum(axis=1).T.reshape(SS)
        sl = slice(SS * c, SS * (c + 1))
        L = -0.5 * q_sum / SD_VAR + T * C0
        ll_last = -0.5 * q_last / SD_VAR + C0
        A_sum = (R_all[sl] + rlast_all[sl]
                 - ALPHA * (L + ll_last) - T * mx_pos)
        total += np.sum(A_sum * L)
    out = np.float32(total / S)
    if _run_kwargs:
        _NC_CACHE["last_result"] = res
    return out


if __name__ == "__main__":
    rng = np.random.default_rng(0)
    inputs = {
        "states": rng.standard_normal((S, D, T), dtype=np.float32),
        "actions": rng.standard_normal((S, A, T), dtype=np.float32),
        "rewards": rng.standard_normal((S, T), dtype=np.float32),
        "W1": (rng.standard_normal((D, HID)) / np.sqrt(D)).astype(np.float32),
        "b1": np.zeros(HID, np.float32),
        "W2": (rng.standard_normal((HID, A)) / np.sqrt(HID)).astype(np.float32),
        "b2": np.zeros(A, np.float32),
    }
    print("result:", kernel(**inputs))
